# revision 24
# baseline (speedup 1.0000x reference)
"""Trainium2 Bass kernel for nn_InvKin: 4-layer MLP (3->1024->1024->1024->3)
with full-batch BatchNorm + ReLU, followed by a closed-form 3-joint forward
kinematics model. Data-parallel over 8 NeuronCores.

v2 structure (vs v1 baseline):
- Layer-1 BN stats come from exact x-moments computed REDUNDANTLY on every
  core from the full (replicated, 1MB fp16) input via DVE bn_stats -- no
  AllReduce and no dependency on the collectives entry barrier, so the PE
  starts layer-1/2 work at ~20us instead of ~86us.
- Everything is processed in PAIRS of 512-row batch blocks: 2-bank PSUM
  accumulator tiles, [128, MC, 2, NB] activation tiles, one cast / one
  activation / one bn_stats / one DMA per (pair, chunk) instead of two.
  Casts split across DVE+ACT engines, applies split, stats on DVE.
- AllReduces for layer-2/3 stats keep their small DMAs on the vector-engine
  queue; z reload DMAs ride the scalar-engine queue with the first two pairs
  pre-issued before stats finalize so reloads overlap the AllReduce.
- Matmuls fp16 (fp32 PSUM); stats and normalization fp32. Linear biases are
  absorbed into the BN affine terms (they cancel).

Batch is PERMUTED on host (column j of a shard holds shard row
(j%128)*128 + j//128) so the final per-128-col PE transposes of theta land
partition p at output row p*128+c, making theta/pred stores fully contiguous.
"""
import sys

sys.path.insert(0, "/opt/trn_rl_repo")

import numpy as np

import concourse.bass as bass
import concourse.tile as tile
from concourse import bacc, mybir
from concourse.bass_utils import run_bass_kernel_spmd

N_CORES = 8
B = 131072
BL = B // N_CORES          # rows per core
H = 1024
NB = 512                   # batch rows per block (one PSUM bank of fp32)
NBLK = BL // NB            # 32 blocks per core
NPAIR = NBLK // 2          # 16 pairs per core
MC = H // 128              # 128-feature chunks per layer
NT = BL // 128             # 128 column-chunks per core
BN_EPS = 1e-5
PI = float(np.pi)

F16 = mybir.dt.float16
F32 = mybir.dt.float32
AF = mybir.ActivationFunctionType
ALU = mybir.AluOpType

_MODULE = None


def _build_module(bl=BL, ncores=N_CORES):
    Bt = bl * ncores
    npair = bl // (2 * NB)
    nt = bl // 128
    nc = bacc.Bacc("TRN2", target_bir_lowering=False, debug=False,
                   num_devices=ncores)

    # ---- I/O ----
    xt_in = nc.dram_tensor("xt", [3, bl], F16, kind="ExternalInput").ap()
    # full-batch x (+ ones col) packed [128, 2, Bt/256, 4] for moments
    xf_in = nc.dram_tensor("xf4", [128, 2, Bt // 256, 4], F16,
                           kind="ExternalInput").ap()
    w1h_in = nc.dram_tensor("w1h", [3, H], F16, kind="ExternalInput").ap()
    w1f_in = nc.dram_tensor("w1f", [3, H], F32, kind="ExternalInput").ap()
    # host pre-arranged [128, MC, H] so loads are contiguous
    w2h_in = nc.dram_tensor("w2h", [128, MC, H], F16, kind="ExternalInput").ap()
    w3h_in = nc.dram_tensor("w3h", [128, MC, H], F16, kind="ExternalInput").ap()
    w4h_in = nc.dram_tensor("w4h", [128, MC, 3], F16, kind="ExternalInput").ap()
    g1_in = nc.dram_tensor("g1v", [128, MC], F32, kind="ExternalInput").ap()
    bt1_in = nc.dram_tensor("bt1v", [128, MC], F32, kind="ExternalInput").ap()
    g2_in = nc.dram_tensor("g2v", [128, MC], F32, kind="ExternalInput").ap()
    bt2_in = nc.dram_tensor("bt2v", [128, MC], F32, kind="ExternalInput").ap()
    g3_in = nc.dram_tensor("g3v", [128, MC], F32, kind="ExternalInput").ap()
    bt3_in = nc.dram_tensor("bt3v", [128, MC], F32, kind="ExternalInput").ap()
    b4_in = nc.dram_tensor("b4v", [3, 1], F32, kind="ExternalInput").ap()
    eye3_in = nc.dram_tensor("eye3", [3, 3], F32, kind="ExternalInput").ap()

    theta_out = nc.dram_tensor("theta", [bl, 3], F32, kind="ExternalOutput").ap()
    pred_out = nc.dram_tensor("pred", [bl, 3], F32, kind="ExternalOutput").ap()

    NF = Bt // 256          # 512 free cols per bn_stats half

    with tile.TileContext(nc) as tc:
        with tc.tile_pool(name="wp", bufs=1) as wp, \
             tc.tile_pool(name="sp", bufs=1) as sp, \
             tc.tile_pool(name="xp", bufs=2) as xp, \
             tc.tile_pool(name="hp", bufs=3) as hp, \
             tc.tile_pool(name="zlp", bufs=3) as zlp, \
             tc.tile_pool(name="zep", bufs=2) as zep, \
             tc.tile_pool(name="wq", bufs=1) as wq, \
             tc.tile_pool(name="stp", bufs=1) as stp, \
             tc.tile_pool(name="psA", bufs=2, space="PSUM") as psA, \
             tc.tile_pool(name="psB", bufs=2, space="PSUM") as psB, \
             tc.tile_pool(name="dr", bufs=1, space="DRAM") as dr:

            # ---- dummy AllReduce at t~0: absorbs the core-start skew on the
            # idle gpsimd engine while the PE computes, so the first REAL
            # collective (AR2 at ~700us) finds all cores already in lockstep.
            dz = wp.tile([1, 1], F32)
            nc.vector.memset(dz, 0.0)
            dummy_i = dr.tile([1, 1], F32, name="dummy_i")
            dummy_o = dr.tile([1, 1], F32, name="dummy_o")
            nc.gpsimd.dma_start(out=dummy_i, in_=dz)
            nc.gpsimd.collective_compute(
                "AllReduce", ALU.add,
                replica_groups=[list(range(ncores))],
                ins=[dummy_i.opt()], outs=[dummy_o.opt()],
            )

            # ---- critical inputs first on the sync DMA queue ----
            # xf borrows a zep ring slot (phase A finishes early, slot recycles)
            xf = zep.tile([128, 2, NF, 4], F16, tag="ze", name="xf")
            nc.sync.dma_start(out=xf, in_=xf_in)
            w1s = wp.tile([3, H], F16)
            nc.sync.dma_start(out=w1s, in_=w1h_in)
            w1fs = wp.tile([3, H], F32)
            nc.sync.dma_start(out=w1fs, in_=w1f_in)
            xt3 = xt_in.rearrange("a (b c) -> a b c", c=NB)

            w2s = wq.tile([128, MC, H], F16, tag="w", name="w2s")
            nc.sync.dma_start(out=w2s, in_=w2h_in)
            w4s = wp.tile([128, MC, 3], F16)
            nc.sync.dma_start(out=w4s, in_=w4h_in)

            def load_param(ap_in, name):
                t = wp.tile([128, MC], F32, name=name)
                nc.sync.dma_start(out=t, in_=ap_in)
                return t

            g1s = load_param(g1_in, "g1s")
            bt1s = load_param(bt1_in, "bt1s")
            g2s = load_param(g2_in, "g2s")
            bt2s = load_param(bt2_in, "bt2s")
            g3s = load_param(g3_in, "g3s")
            bt3s = load_param(bt3_in, "bt3s")
            b4s = wp.tile([3, 1], F32)
            nc.sync.dma_start(out=b4s, in_=b4_in)
            eye3s = wp.tile([3, 3], F32)
            nc.sync.dma_start(out=eye3s, in_=eye3_in)
            eye3h = wp.tile([3, 3], F16)
            nc.vector.tensor_copy(out=eye3h, in_=eye3s)

            eps_t = wp.tile([128, 1], F32)
            nc.vector.memset(eps_t, BN_EPS)
            zero128 = wp.tile([128, 1], F32)
            nc.vector.memset(zero128, 0.0)
            ones3 = wp.tile([3, 1], F32)
            nc.vector.memset(ones3, 1.0)
            inv128 = wp.tile([128, 1], F32)
            nc.vector.memset(inv128, 1.0 / 128.0)

            # ---- DRAM intermediates ----
            z2buf = dr.tile([128, npair, MC, 2, NB], F16)
            z3buf = dr.tile([128, npair, MC, 2, NB], F16)
            m12d = dr.tile([12, 1], F32)

            thn = wp.tile([128, nt, 3], F32)

            # =========================================================
            # Phase A: full-batch x moments on DVE (no collective).
            # partials12[:, 0:3]  = per-partition E[x_a]
            # partials12[:, 3:12] = per-partition E[x_a x_b] (3x3 rowmajor)
            # =========================================================
            partials12 = sp.tile([128, 12], F32)
            aggv = sp.tile([128, 2], F32, name="aggv")
            tmp1 = sp.tile([128, 1], F32, name="tmp1")
            stt = sp.tile([128, 2, 6], F32, name="stt")
            for a in range(3):
                for hh in range(2):
                    nc.vector.bn_stats(out=stt[:, hh], in_=xf[:, hh, :, a])
                nc.vector.bn_aggr(out=aggv, in_=stt)
                nc.vector.tensor_copy(out=partials12[:, a:a + 1],
                                      in_=aggv[:, 0:1])
                # E[x^2] = var + mean^2
                nc.vector.tensor_mul(tmp1, aggv[:, 0:1], aggv[:, 0:1])
                nc.vector.tensor_add(partials12[:, 3 + 4 * a:4 + 4 * a],
                                     aggv[:, 1:2], tmp1)
            prodt = sp.tile([128, 2, NF], F32, name="prodt")
            for (a, b2, cols) in ((0, 1, (1, 3)), (0, 2, (2, 6)), (1, 2, (5, 7))):
                nc.vector.tensor_mul(prodt, xf[:, :, :, a], xf[:, :, :, b2])
                for hh in range(2):
                    nc.vector.bn_stats(out=stt[:, hh], in_=prodt[:, hh])
                nc.vector.bn_aggr(out=aggv, in_=stt)
                for c in cols:
                    nc.vector.tensor_copy(out=partials12[:, 3 + c:4 + c],
                                          in_=aggv[:, 0:1])
            momp = psA.tile([12, 1], F32, tag="pA", name="momp")
            nc.tensor.matmul(momp[:], partials12, inv128, start=True, stop=True)
            moms = sp.tile([12, 1], F32)
            nc.vector.tensor_copy(out=moms, in_=momp)
            nc.scalar.dma_start(out=m12d, in_=moms)
            mxs = sp.tile([3, 1], F32)
            nc.scalar.dma_start(out=mxs, in_=m12d[0:3])
            m2s = sp.tile([3, 3], F32)
            nc.scalar.dma_start(out=m2s,
                                in_=m12d[3:12].rearrange("(a b) o -> a (b o)", b=3))

            # mw[p, m] = (mean_x @ W1); q[p, m] = E[(x.w)^2]
            mw = sp.tile([128, MC], F32)
            for m in range(MC):
                pp = psA.tile([128, 1], F32, tag="pA", name=f"mwp{m}")
                nc.tensor.matmul(pp[:], w1fs[:, m * 128:(m + 1) * 128], mxs,
                                 start=True, stop=True)
                nc.vector.tensor_copy(out=mw[:, m:m + 1], in_=pp)
            Asb = sp.tile([3, H], F32)
            for hf in range(2):
                ap_ = psA.tile([3, 512], F32, tag="pA", name=f"Ap{hf}")
                nc.tensor.matmul(ap_[:], m2s, w1fs[:, hf * 512:(hf + 1) * 512],
                                 start=True, stop=True)
                nc.vector.tensor_copy(out=Asb[:, hf * 512:(hf + 1) * 512], in_=ap_)
            Psb = sp.tile([3, H], F32)
            nc.vector.tensor_mul(Psb, w1fs, Asb)
            q = sp.tile([128, MC], F32)
            for m in range(MC):
                pp2 = psA.tile([128, 1], F32, tag="pA", name=f"qp{m}")
                nc.tensor.matmul(pp2[:], Psb[:, m * 128:(m + 1) * 128], ones3,
                                 start=True, stop=True)
                nc.vector.tensor_copy(out=q[:, m:m + 1], in_=pp2)

            v1t = sp.tile([128, MC], F32)
            nc.vector.tensor_mul(v1t, mw, mw)
            nc.vector.tensor_sub(v1t, q, v1t)
            sd1 = sp.tile([128, MC], F32)
            nc.scalar.activation(out=sd1, in_=v1t, func=AF.Sqrt, bias=eps_t[:])
            rstd1 = sp.tile([128, MC], F32)
            nc.vector.reciprocal(out=rstd1, in_=sd1)
            s1 = sp.tile([128, MC], F32)
            nc.vector.tensor_mul(s1, g1s, rstd1)
            t1p = sp.tile([128, MC], F32)
            nc.vector.tensor_mul(t1p, mw, s1)
            nc.vector.tensor_sub(t1p, bt1s, t1p)

            # =========================================================
            # Shared helpers
            # =========================================================
            def apply_pair(h, m, zsrc, s_, t_, veng):
                """h[:, m] = relu(s*z + t) over a [128, 2, NB] pair slice.
                s_ None means the scale is folded into the next layer's
                weights: relu(z + t) in one DVE op."""
                if veng:
                    if s_ is None:
                        # folded form: one op; route one chunk to the idle
                        # gpsimd engine to relieve DVE/ACT in layer 4
                        eng = nc.gpsimd if m == 4 else nc.vector
                        eng.tensor_scalar(
                            out=h[:, m], in0=zsrc,
                            scalar1=t_[:, m:m + 1], scalar2=0.0,
                            op0=ALU.add, op1=ALU.max)
                    else:
                        nc.vector.tensor_scalar(
                            out=h[:, m], in0=zsrc,
                            scalar1=s_[:, m:m + 1], scalar2=t_[:, m:m + 1],
                            op0=ALU.mult, op1=ALU.add)
                        nc.vector.tensor_scalar_max(h[:, m], h[:, m], 0.0)
                else:
                    nc.scalar.activation(
                        out=h[:, m], in_=zsrc, func=AF.Relu,
                        bias=t_[:, m:m + 1],
                        scale=(1.0 if s_ is None else s_[:, m:m + 1]))

            def l1_pair(p):
                """Fused layer 1 for pair p: z1 = x@W1 in PSUM, BN+relu apply
                straight to an h pair tile."""
                h = hp.tile([128, MC, 2, NB], F16, tag="h", name=f"h1_{p}")
                xtb = xp.tile([3, 2, NB], F16, tag="xtb", name=f"xtb{p}")
                nc.sync.dma_start(out=xtb, in_=xt3[:, 2 * p:2 * p + 2])
                for m in range(MC):
                    zp = psA.tile([128, 2, NB], F32, tag="pA", name=f"z1_{p}_{m}")
                    nc.tensor.matmul(zp[:, 0], w1s[:, m * 128:(m + 1) * 128],
                                     xtb[:, 0], start=True, stop=True)
                    nc.tensor.matmul(zp[:, 1], w1s[:, m * 128:(m + 1) * 128],
                                     xtb[:, 1], start=True, stop=True)
                    apply_pair(h, m, zp, s1, t1p, veng=(m < 3))
                return h

            def mm_pair(win, h, stats, zdst, p, nm, last=False):
                """One pair of batch blocks through W; casts split ACT/DVE,
                stats on DVE; one fat store DMA per pair. For the LAST pair
                all casts go to ACT so the DVE finishes stats with the PE --
                shortens the pre-AllReduce tail."""
                zeP = zep.tile([128, MC, 2, NB], F16, tag="ze", name=f"ze{nm}_{p}")
                for m2 in range(MC):
                    acc = psB.tile([128, 2, NB], F32, tag="pB",
                                   name=f"z{nm}_{p}_{m2}")
                    for k in range(MC):
                        w_km = win[:, k, m2 * 128:(m2 + 1) * 128]
                        nc.tensor.matmul(acc[:, 0], w_km, h[:, k, 0],
                                         start=(k == 0), stop=(k == MC - 1))
                        nc.tensor.matmul(acc[:, 1], w_km, h[:, k, 1],
                                         start=(k == 0), stop=(k == MC - 1))
                    if m2 % 2 == 0 and not last:
                        nc.vector.tensor_copy(out=zeP[:, m2], in_=acc)
                    else:
                        nc.scalar.copy(out=zeP[:, m2], in_=acc)
                    nc.vector.bn_stats(out=stats[:, m2, p, 0], in_=zeP[:, m2, 0])
                    nc.vector.bn_stats(out=stats[:, m2, p, 1], in_=zeP[:, m2, 1])
                nc.sync.dma_start(out=zdst[:, p], in_=zeP)

            def zl_load(zsrc, p, nm):
                zl = zlp.tile([128, MC, 2, NB], F16, tag="zl",
                              name=f"zl{nm}_{p}")
                # alternate DMA queues so reloads aren't serialized on one
                (nc.scalar if p % 2 == 0 else nc.gpsimd).dma_start(
                    out=zl, in_=zsrc[:, p])
                return zl

            def h_from_z(zsrc, s_, t_, p, nm, zl_pre=None, veng_n=3):
                zl = zl_pre if zl_pre is not None else zl_load(zsrc, p, nm)
                h = hp.tile([128, MC, 2, NB], F16, tag="h", name=f"h{nm}_{p}")
                for m in range(MC):
                    apply_pair(h, m, zl[:, m], s_, t_, veng=(m < veng_n))
                return h

            def prefetch_zl(zsrc, nm, n=3):
                return [zl_load(zsrc, p, nm) for p in range(n)]

            def half_aggr(stats, nm):
                """Aggregate the first npair/2 pairs' records early (free --
                runs while the second half still computes)."""
                mvA = sp.tile([128, MC, 2], F32, name=f"mvA{nm}")
                for m in range(MC):
                    nc.vector.bn_aggr(out=mvA[:, m], in_=stats[:, m, :npair // 2])
                return mvA

            def finalize_stats(stats, g_s, bt_s, nm, mvA):
                mvB = sp.tile([128, MC, 2], F32, name=f"mvB{nm}")
                for m in range(MC):
                    nc.vector.bn_aggr(out=mvB[:, m], in_=stats[:, m, npair // 2:])
                # per-half counts are equal (bl/2): S1 = (bl/2)(mA+mB),
                # S2 = (bl/2)(vA+mA^2 + vB+mB^2)
                cci = sp.tile([128, MC, 2], F32, name=f"cci{nm}")
                tmp = sp.tile([128, MC], F32, name=f"tmq{nm}")
                tmpb = sp.tile([128, MC], F32, name=f"tmqb{nm}")
                nc.vector.tensor_mul(tmp, mvA[:, :, 0], mvA[:, :, 0])
                nc.vector.tensor_add(tmp, tmp, mvA[:, :, 1])
                nc.vector.tensor_mul(tmpb, mvB[:, :, 0], mvB[:, :, 0])
                nc.vector.tensor_add(tmpb, tmpb, mvB[:, :, 1])
                nc.vector.tensor_add(tmp, tmp, tmpb)
                nc.vector.tensor_scalar_mul(cci[:, :, 1], tmp, float(bl) / 2)
                nc.vector.tensor_add(tmpb, mvA[:, :, 0], mvB[:, :, 0])
                nc.vector.tensor_scalar_mul(cci[:, :, 0], tmpb, float(bl) / 2)
                di = dr.tile([128, MC * 2], F32, name=f"di{nm}")
                do_ = dr.tile([128, MC * 2], F32, name=f"do{nm}")
                nc.gpsimd.dma_start(out=di, in_=cci)
                nc.gpsimd.collective_compute(
                    "AllReduce", ALU.add,
                    replica_groups=[list(range(ncores))],
                    ins=[di.opt()], outs=[do_.opt()],
                )
                ccg = sp.tile([128, MC, 2], F32, name=f"ccg{nm}")
                nc.gpsimd.dma_start(out=ccg, in_=do_)
                meanv = sp.tile([128, MC], F32, name=f"mean{nm}")
                nc.vector.tensor_scalar_mul(meanv, ccg[:, :, 0], 1.0 / Bt)
                ex2 = sp.tile([128, MC], F32, name=f"ex2{nm}")
                nc.vector.tensor_scalar_mul(ex2, ccg[:, :, 1], 1.0 / Bt)
                vart = sp.tile([128, MC], F32, name=f"var{nm}")
                nc.vector.tensor_mul(vart, meanv, meanv)
                nc.vector.tensor_sub(vart, ex2, vart)
                sd = sp.tile([128, MC], F32, name=f"sd{nm}")
                nc.scalar.activation(out=sd, in_=vart, func=AF.Sqrt, bias=eps_t[:])
                rstd = sp.tile([128, MC], F32, name=f"rstd{nm}")
                nc.vector.reciprocal(out=rstd, in_=sd)
                s_ = sp.tile([128, MC], F32, name=f"s{nm}")
                nc.vector.tensor_mul(s_, g_s, rstd)
                t_ = sp.tile([128, MC], F32, name=f"t{nm}")
                nc.vector.tensor_mul(t_, meanv, s_)
                nc.vector.tensor_sub(t_, bt_s, t_)
                return s_, t_

            # =========================================================
            # Layers 1+2 (software-pipelined: l1 runs 2 pairs ahead)
            # =========================================================
            st2 = stp.tile([128, MC, npair, 2, 6], F32, tag="st", name="st2")
            hq = [l1_pair(0), l1_pair(1)]
            mvA2 = None
            for p in range(npair):
                if p + 2 < npair:
                    hq.append(l1_pair(p + 2))
                mm_pair(w2s, hq[p], st2, z2buf, p, "2", last=(p == npair - 1))
                if p == npair // 2 - 1:
                    mvA2 = half_aggr(st2, "2")
            pre3 = prefetch_zl(z2buf, "3")
            # W3 reuses W2's buffer; its load lands in the AR2 window
            w3s = wq.tile([128, MC, H], F16, tag="w", name="w3s")
            nc.sync.dma_start(out=w3s, in_=w3h_in)
            s2, t2p = finalize_stats(st2, g2s, bt2s, "2", mvA2)

            # ---- Layer 3 ----
            st3 = stp.tile([128, MC, npair, 2, 6], F32, tag="st", name="st3")
            mvA3 = None
            for p in range(npair):
                h = h_from_z(z2buf, s2, t2p, p, "3",
                             zl_pre=pre3[p] if p < len(pre3) else None)
                mm_pair(w3s, h, st3, z3buf, p, "3", last=(p == npair - 1))
                if p == npair // 2 - 1:
                    mvA3 = half_aggr(st3, "3")
            pre4 = prefetch_zl(z3buf, "4")
            s3, t3p = finalize_stats(st3, g3s, bt3s, "3", mvA3)
            # fold s3 into W4 rows: h3' = relu(z3 + t3/s3), W4' = s3*W4
            rs3 = sp.tile([128, MC], F32)
            nc.vector.reciprocal(out=rs3, in_=s3)
            t3d = sp.tile([128, MC], F32)
            nc.vector.tensor_mul(t3d, t3p, rs3)
            for k in range(MC):
                nc.vector.tensor_scalar_mul(w4s[:, k], w4s[:, k], s3[:, k:k + 1])

            # =========================================================
            # Layer 4 -> theta, transposed on-chip via PE
            # =========================================================
            for p in range(npair):
                h3 = h_from_z(z3buf, None, t3d, p, "4",
                              zl_pre=pre4[p] if p < len(pre4) else None,
                              veng_n=5)
                thp = psB.tile([3, 2, NB], F32, tag="pB", name=f"thp{p}")
                for k in range(MC):
                    nc.tensor.matmul(thp[:, 0], w4s[:, k], h3[:, k, 0],
                                     start=(k == 0), stop=(k == MC - 1))
                    nc.tensor.matmul(thp[:, 1], w4s[:, k], h3[:, k, 1],
                                     start=(k == 0), stop=(k == MC - 1))
                ths = xp.tile([3, 2, NB], F16, tag="ths", name=f"ths{p}")
                nc.scalar.activation(out=ths, in_=thp, func=AF.Identity,
                                     bias=b4s[:], scale=1.0)
                for j in range(8):
                    tps = psA.tile([128, 3], F16, tag="pA", name=f"tps{p}_{j}")
                    nc.tensor.transpose(
                        tps[:], ths[:, j // 4, (j % 4) * 128:(j % 4 + 1) * 128],
                        eye3h)
                    nc.vector.tensor_copy(out=thn[:, 8 * p + j, :], in_=tps)
            nc.sync.dma_start(
                out=theta_out.rearrange("(p t) f -> p t f", p=128), in_=thn)

            # =========================================================
            # Forward kinematics on thn (batch on partitions x nt free)
            # =========================================================
            def trig(src, shift, nm):
                w = sp.tile([128, nt], F32, name=f"w{nm}")
                nc.vector.add_range_wrap(out=w, in_=src, shift=shift,
                                         bound=PI, period=2 * PI)
                o = sp.tile([128, nt], F32, name=f"o{nm}")
                nc.scalar.activation(out=o, in_=w, func=AF.Sin, bias=zero128[:])
                return o

            th0 = thn[:, :, 0]
            th1 = thn[:, :, 1]
            th2 = thn[:, :, 2]
            t12 = sp.tile([128, nt], F32, name="t12")
            nc.vector.tensor_add(t12, th1, th2)
            s0v = trig(th0, 0.0, "s0")
            c0v = trig(th0, PI / 2, "c0")
            s1v = trig(th1, 0.0, "s1v")
            c1v = trig(th1, PI / 2, "c1v")
            s12v = trig(t12, 0.0, "s12")
            c12v = trig(t12, PI / 2, "c12")

            Lt = sp.tile([128, nt], F32, name="Lt")
            nc.vector.tensor_scalar_mul(Lt, c12v, 0.115)
            nc.vector.scalar_tensor_tensor(out=Lt, in0=c1v, scalar=0.12, in1=Lt,
                                           op0=ALU.mult, op1=ALU.add)
            pzt = sp.tile([128, nt], F32, name="pzt")
            nc.vector.tensor_scalar_mul(pzt, s12v, 0.115)
            nc.vector.scalar_tensor_tensor(out=pzt, in0=s1v, scalar=0.12, in1=pzt,
                                           op0=ALU.mult, op1=ALU.add)
            predn = sp.tile([128, nt, 3], F32, name="predn")
            nc.vector.tensor_mul(predn[:, :, 0], c0v, Lt)
            nc.vector.tensor_mul(predn[:, :, 1], s0v, Lt)
            nc.vector.tensor_copy(out=predn[:, :, 2], in_=pzt)
            nc.sync.dma_start(
                out=pred_out.rearrange("(p t) f -> p t f", p=128), in_=predn)

    nc.compile()
    return nc


def _get_module():
    global _MODULE
    if _MODULE is None:
        _MODULE = _build_module()
    return _MODULE


def kernel(x, W1, b1, g1, bt1, W2, b2, g2, bt2, W3, b3, g3, bt3, W4, b4,
           **run_kwargs):
    nc = _get_module()
    x = np.asarray(x, dtype=np.float32)
    x16 = x.astype(np.float16)
    # full batch + ones col, packed [128, 2, B/256, 4] (any row->slot
    # assignment works -- only sums are taken)
    xa_full = np.concatenate(
        [x16, np.ones((x16.shape[0], 1), np.float16)], axis=1)
    xf4 = np.ascontiguousarray(
        xa_full.reshape(2, B // 256, 128, 4).transpose(2, 0, 1, 3))

    def prearr_w(W):
        W = np.asarray(W, np.float32).astype(np.float16)
        return np.ascontiguousarray(
            W.reshape(MC, 128, -1).transpose(1, 0, 2))

    def prearr_p(v):
        return np.ascontiguousarray(
            np.asarray(v, np.float32).reshape(MC, 128).T)

    shared = {
        "xf4": xf4,
        "w1h": np.ascontiguousarray(np.asarray(W1, np.float32).astype(np.float16)),
        "w1f": np.ascontiguousarray(np.asarray(W1, np.float32)),
        "w2h": prearr_w(W2),
        "w3h": prearr_w(W3),
        "w4h": prearr_w(W4),
        "g1v": prearr_p(g1),
        "bt1v": prearr_p(bt1),
        "g2v": prearr_p(g2),
        "bt2v": prearr_p(bt2),
        "g3v": prearr_p(g3),
        "bt3v": prearr_p(bt3),
        "b4v": np.ascontiguousarray(np.asarray(b4, np.float32).reshape(3, 1)),
        "eye3": np.eye(3, dtype=np.float32),
    }
    in_maps = []
    for i in range(N_CORES):
        xs = x[i * BL:(i + 1) * BL]
        # permuted transposed shard: column c*128+p holds shard row p*128+c
        xt_p = xs.T.astype(np.float16).reshape(3, 128, BL // 128) \
            .swapaxes(1, 2).reshape(3, BL)
        m = dict(shared)
        m["xt"] = np.ascontiguousarray(xt_p)
        in_maps.append(m)
    res = run_bass_kernel_spmd(nc, in_maps, core_ids=list(range(N_CORES)),
                               **run_kwargs)
    theta = np.concatenate([res.results[i]["theta"] for i in range(N_CORES)], axis=0)
    pred = np.concatenate([res.results[i]["pred"] for i in range(N_CORES)], axis=0)
    kernel.last_results = res
    return theta.astype(np.float32), pred.astype(np.float32)


# revision 25
# speedup vs baseline: 1.1698x; 1.1698x over previous
"""Trainium2 Bass kernel for nn_InvKin: 4-layer MLP (3->1024->1024->1024->3)
with full-batch BatchNorm + ReLU, followed by a closed-form 3-joint forward
kinematics model. Data-parallel over 8 NeuronCores.

v2 structure (vs v1 baseline):
- Layer-1 BN stats come from exact x-moments computed REDUNDANTLY on every
  core from the full (replicated, 1MB fp16) input via DVE bn_stats -- no
  AllReduce and no dependency on the collectives entry barrier, so the PE
  starts layer-1/2 work at ~20us instead of ~86us.
- Everything is processed in PAIRS of 512-row batch blocks: 2-bank PSUM
  accumulator tiles, [128, MC, 2, NB] activation tiles, one cast / one
  activation / one bn_stats / one DMA per (pair, chunk) instead of two.
  Casts split across DVE+ACT engines, applies split, stats on DVE.
- AllReduces for layer-2/3 stats keep their small DMAs on the vector-engine
  queue; z reload DMAs ride the scalar-engine queue with the first two pairs
  pre-issued before stats finalize so reloads overlap the AllReduce.
- Matmuls fp16 (fp32 PSUM); stats and normalization fp32. Linear biases are
  absorbed into the BN affine terms (they cancel).

Batch is PERMUTED on host (column j of a shard holds shard row
(j%128)*128 + j//128) so the final per-128-col PE transposes of theta land
partition p at output row p*128+c, making theta/pred stores fully contiguous.
"""
import sys

sys.path.insert(0, "/opt/trn_rl_repo")

import numpy as np

import concourse.bass as bass
import concourse.tile as tile
from concourse import bacc, mybir
from concourse.bass_utils import run_bass_kernel_spmd

N_CORES = 8
B = 131072
BL = B // N_CORES          # rows per core
H = 1024
NB = 512                   # batch rows per block (one PSUM bank of fp32)
NBLK = BL // NB            # 32 blocks per core
NPAIR = NBLK // 2          # 16 pairs per core
MC = H // 128              # 128-feature chunks per layer
NT = BL // 128             # 128 column-chunks per core
BN_EPS = 1e-5
PI = float(np.pi)

F16 = mybir.dt.float16
F32 = mybir.dt.float32
AF = mybir.ActivationFunctionType
ALU = mybir.AluOpType

_MODULE = None


def _build_module(bl=BL, ncores=N_CORES):
    Bt = bl * ncores
    npair = bl // (2 * NB)
    nt = bl // 128
    nc = bacc.Bacc("TRN2", target_bir_lowering=False, debug=False,
                   num_devices=ncores)

    # ---- I/O ----
    xt_in = nc.dram_tensor("xt", [3, bl], F16, kind="ExternalInput").ap()
    # full-batch x (+ ones col) packed [128, 2, Bt/256, 4] for moments
    xf_in = nc.dram_tensor("xf4", [128, 2, Bt // 256, 4], F16,
                           kind="ExternalInput").ap()
    w1h_in = nc.dram_tensor("w1h", [3, H], F16, kind="ExternalInput").ap()
    w1f_in = nc.dram_tensor("w1f", [3, H], F32, kind="ExternalInput").ap()
    # host pre-arranged [128, MC, H] so loads are contiguous
    w2h_in = nc.dram_tensor("w2h", [128, MC, H], F16, kind="ExternalInput").ap()
    w3h_in = nc.dram_tensor("w3h", [128, MC, H], F16, kind="ExternalInput").ap()
    w4h_in = nc.dram_tensor("w4h", [128, MC, 3], F16, kind="ExternalInput").ap()
    g1_in = nc.dram_tensor("g1v", [128, MC], F32, kind="ExternalInput").ap()
    bt1_in = nc.dram_tensor("bt1v", [128, MC], F32, kind="ExternalInput").ap()
    g2_in = nc.dram_tensor("g2v", [128, MC], F32, kind="ExternalInput").ap()
    bt2_in = nc.dram_tensor("bt2v", [128, MC], F32, kind="ExternalInput").ap()
    g3_in = nc.dram_tensor("g3v", [128, MC], F32, kind="ExternalInput").ap()
    bt3_in = nc.dram_tensor("bt3v", [128, MC], F32, kind="ExternalInput").ap()
    b4_in = nc.dram_tensor("b4v", [3, 1], F32, kind="ExternalInput").ap()
    eye3_in = nc.dram_tensor("eye3", [3, 3], F32, kind="ExternalInput").ap()

    theta_out = nc.dram_tensor("theta", [bl, 3], F32, kind="ExternalOutput").ap()
    pred_out = nc.dram_tensor("pred", [bl, 3], F32, kind="ExternalOutput").ap()

    NF = Bt // 256          # 512 free cols per bn_stats half

    with tile.TileContext(nc) as tc:
        with tc.tile_pool(name="wp", bufs=1) as wp, \
             tc.tile_pool(name="sp", bufs=1) as sp, \
             tc.tile_pool(name="xp", bufs=2) as xp, \
             tc.tile_pool(name="hp", bufs=3) as hp, \
             tc.tile_pool(name="zlp", bufs=3) as zlp, \
             tc.tile_pool(name="zep", bufs=2) as zep, \
             tc.tile_pool(name="wq", bufs=1) as wq, \
             tc.tile_pool(name="stp", bufs=1) as stp, \
             tc.tile_pool(name="psA", bufs=2, space="PSUM") as psA, \
             tc.tile_pool(name="psB", bufs=2, space="PSUM") as psB, \
             tc.tile_pool(name="dr", bufs=1, space="DRAM") as dr:

            # ---- dummy AllReduce at t~0: absorbs the core-start skew on the
            # idle gpsimd engine while the PE computes, so the first REAL
            # collective (AR2 at ~700us) finds all cores already in lockstep.
            dz = wp.tile([1, 1], F32)
            nc.vector.memset(dz, 0.0)
            dummy_i = dr.tile([1, 1], F32, name="dummy_i")
            dummy_o = dr.tile([1, 1], F32, name="dummy_o")
            nc.gpsimd.dma_start(out=dummy_i, in_=dz)
            nc.gpsimd.collective_compute(
                "AllReduce", ALU.add,
                replica_groups=[list(range(ncores))],
                ins=[dummy_i.opt()], outs=[dummy_o.opt()],
            )

            # ---- critical inputs first on the sync DMA queue ----
            # xf borrows a zep ring slot (phase A finishes early, slot recycles)
            xf = zep.tile([128, 2, NF, 4], F16, tag="ze", name="xf")
            nc.sync.dma_start(out=xf, in_=xf_in)
            w1s = wp.tile([3, H], F16)
            nc.sync.dma_start(out=w1s, in_=w1h_in)
            w1fs = wp.tile([3, H], F32)
            nc.sync.dma_start(out=w1fs, in_=w1f_in)
            xt3 = xt_in.rearrange("a (b c) -> a b c", c=NB)

            w2s = wq.tile([128, MC, H], F16, tag="w", name="w2s")
            nc.sync.dma_start(out=w2s, in_=w2h_in)
            w4s = wp.tile([128, MC, 3], F16)
            nc.sync.dma_start(out=w4s, in_=w4h_in)

            def load_param(ap_in, name):
                t = wp.tile([128, MC], F32, name=name)
                nc.sync.dma_start(out=t, in_=ap_in)
                return t

            g1s = load_param(g1_in, "g1s")
            bt1s = load_param(bt1_in, "bt1s")
            g2s = load_param(g2_in, "g2s")
            bt2s = load_param(bt2_in, "bt2s")
            g3s = load_param(g3_in, "g3s")
            bt3s = load_param(bt3_in, "bt3s")
            b4s = wp.tile([3, 1], F32)
            nc.sync.dma_start(out=b4s, in_=b4_in)
            eye3s = wp.tile([3, 3], F32)
            nc.sync.dma_start(out=eye3s, in_=eye3_in)
            eye3h = wp.tile([3, 3], F16)
            nc.vector.tensor_copy(out=eye3h, in_=eye3s)

            eps_t = wp.tile([128, 1], F32)
            nc.vector.memset(eps_t, BN_EPS)
            zero128 = wp.tile([128, 1], F32)
            nc.vector.memset(zero128, 0.0)
            ones3 = wp.tile([3, 1], F32)
            nc.vector.memset(ones3, 1.0)
            inv128 = wp.tile([128, 1], F32)
            nc.vector.memset(inv128, 1.0 / 128.0)

            # ---- DRAM intermediates ----
            z2buf = dr.tile([128, npair, MC, 2, NB], F16)
            z3buf = dr.tile([128, npair, MC, 2, NB], F16)
            m12d = dr.tile([12, 1], F32)

            thn = wp.tile([128, nt, 3], F32)

            # =========================================================
            # Phase A: full-batch x moments on DVE (no collective).
            # partials12[:, 0:3]  = per-partition E[x_a]
            # partials12[:, 3:12] = per-partition E[x_a x_b] (3x3 rowmajor)
            # =========================================================
            partials12 = sp.tile([128, 12], F32)
            aggv = sp.tile([128, 2], F32, name="aggv")
            tmp1 = sp.tile([128, 1], F32, name="tmp1")
            stt = sp.tile([128, 2, 6], F32, name="stt")
            for a in range(3):
                for hh in range(2):
                    nc.vector.bn_stats(out=stt[:, hh], in_=xf[:, hh, :, a])
                nc.vector.bn_aggr(out=aggv, in_=stt)
                nc.vector.tensor_copy(out=partials12[:, a:a + 1],
                                      in_=aggv[:, 0:1])
                # E[x^2] = var + mean^2
                nc.vector.tensor_mul(tmp1, aggv[:, 0:1], aggv[:, 0:1])
                nc.vector.tensor_add(partials12[:, 3 + 4 * a:4 + 4 * a],
                                     aggv[:, 1:2], tmp1)
            prodt = sp.tile([128, 2, NF], F32, name="prodt")
            for (a, b2, cols) in ((0, 1, (1, 3)), (0, 2, (2, 6)), (1, 2, (5, 7))):
                nc.vector.tensor_mul(prodt, xf[:, :, :, a], xf[:, :, :, b2])
                for hh in range(2):
                    nc.vector.bn_stats(out=stt[:, hh], in_=prodt[:, hh])
                nc.vector.bn_aggr(out=aggv, in_=stt)
                for c in cols:
                    nc.vector.tensor_copy(out=partials12[:, 3 + c:4 + c],
                                          in_=aggv[:, 0:1])
            momp = psA.tile([12, 1], F32, tag="pA", name="momp")
            nc.tensor.matmul(momp[:], partials12, inv128, start=True, stop=True)
            moms = sp.tile([12, 1], F32)
            nc.vector.tensor_copy(out=moms, in_=momp)
            nc.scalar.dma_start(out=m12d, in_=moms)
            mxs = sp.tile([3, 1], F32)
            nc.scalar.dma_start(out=mxs, in_=m12d[0:3])
            m2s = sp.tile([3, 3], F32)
            nc.scalar.dma_start(out=m2s,
                                in_=m12d[3:12].rearrange("(a b) o -> a (b o)", b=3))

            # mw[p, m] = (mean_x @ W1); q[p, m] = E[(x.w)^2]
            mw = sp.tile([128, MC], F32)
            for m in range(MC):
                pp = psA.tile([128, 1], F32, tag="pA", name=f"mwp{m}")
                nc.tensor.matmul(pp[:], w1fs[:, m * 128:(m + 1) * 128], mxs,
                                 start=True, stop=True)
                nc.vector.tensor_copy(out=mw[:, m:m + 1], in_=pp)
            Asb = sp.tile([3, H], F32)
            for hf in range(2):
                ap_ = psA.tile([3, 512], F32, tag="pA", name=f"Ap{hf}")
                nc.tensor.matmul(ap_[:], m2s, w1fs[:, hf * 512:(hf + 1) * 512],
                                 start=True, stop=True)
                nc.vector.tensor_copy(out=Asb[:, hf * 512:(hf + 1) * 512], in_=ap_)
            Psb = sp.tile([3, H], F32)
            nc.vector.tensor_mul(Psb, w1fs, Asb)
            q = sp.tile([128, MC], F32)
            for m in range(MC):
                pp2 = psA.tile([128, 1], F32, tag="pA", name=f"qp{m}")
                nc.tensor.matmul(pp2[:], Psb[:, m * 128:(m + 1) * 128], ones3,
                                 start=True, stop=True)
                nc.vector.tensor_copy(out=q[:, m:m + 1], in_=pp2)

            v1t = sp.tile([128, MC], F32)
            nc.vector.tensor_mul(v1t, mw, mw)
            nc.vector.tensor_sub(v1t, q, v1t)
            sd1 = sp.tile([128, MC], F32)
            nc.scalar.activation(out=sd1, in_=v1t, func=AF.Sqrt, bias=eps_t[:])
            rstd1 = sp.tile([128, MC], F32)
            nc.vector.reciprocal(out=rstd1, in_=sd1)
            s1 = sp.tile([128, MC], F32)
            nc.vector.tensor_mul(s1, g1s, rstd1)
            t1p = sp.tile([128, MC], F32)
            nc.vector.tensor_mul(t1p, mw, s1)
            nc.vector.tensor_sub(t1p, bt1s, t1p)

            # =========================================================
            # Shared helpers
            # =========================================================
            def apply_pair(h, m, zsrc, s_, t_, veng):
                """h[:, m] = relu(s*z + t) over a [128, 2, NB] pair slice.
                s_ None means the scale is folded into the next layer's
                weights: relu(z + t) in one DVE op."""
                if veng:
                    if s_ is None:
                        nc.vector.tensor_scalar(
                            out=h[:, m], in0=zsrc,
                            scalar1=t_[:, m:m + 1], scalar2=0.0,
                            op0=ALU.add, op1=ALU.max)
                    else:
                        nc.vector.tensor_scalar(
                            out=h[:, m], in0=zsrc,
                            scalar1=s_[:, m:m + 1], scalar2=t_[:, m:m + 1],
                            op0=ALU.mult, op1=ALU.add)
                        nc.vector.tensor_scalar_max(h[:, m], h[:, m], 0.0)
                else:
                    nc.scalar.activation(
                        out=h[:, m], in_=zsrc, func=AF.Relu,
                        bias=t_[:, m:m + 1],
                        scale=(1.0 if s_ is None else s_[:, m:m + 1]))

            def l1_pair(p):
                """Fused layer 1 for pair p: z1 = x@W1 in PSUM, BN+relu apply
                straight to an h pair tile."""
                h = hp.tile([128, MC, 2, NB], F16, tag="h", name=f"h1_{p}")
                xtb = xp.tile([3, 2, NB], F16, tag="xtb", name=f"xtb{p}")
                nc.sync.dma_start(out=xtb, in_=xt3[:, 2 * p:2 * p + 2])
                for m in range(MC):
                    zp = psA.tile([128, 2, NB], F32, tag="pA", name=f"z1_{p}_{m}")
                    nc.tensor.matmul(zp[:, 0], w1s[:, m * 128:(m + 1) * 128],
                                     xtb[:, 0], start=True, stop=True)
                    nc.tensor.matmul(zp[:, 1], w1s[:, m * 128:(m + 1) * 128],
                                     xtb[:, 1], start=True, stop=True)
                    apply_pair(h, m, zp, s1, t1p, veng=(m < 3))
                return h

            def mm_pair(win, h, stats, zdst, p, nm, last=False):
                """One pair of batch blocks through W; casts split ACT/DVE,
                stats on DVE; one fat store DMA per pair. For the LAST pair
                all casts go to ACT so the DVE finishes stats with the PE --
                shortens the pre-AllReduce tail."""
                zeP = zep.tile([128, MC, 2, NB], F16, tag="ze", name=f"ze{nm}_{p}")
                for m2 in range(MC):
                    acc = psB.tile([128, 2, NB], F32, tag="pB",
                                   name=f"z{nm}_{p}_{m2}")
                    for k in range(MC):
                        w_km = win[:, k, m2 * 128:(m2 + 1) * 128]
                        nc.tensor.matmul(acc[:, 0], w_km, h[:, k, 0],
                                         start=(k == 0), stop=(k == MC - 1))
                        nc.tensor.matmul(acc[:, 1], w_km, h[:, k, 1],
                                         start=(k == 0), stop=(k == MC - 1))
                    if m2 % 2 == 0 and not last:
                        nc.vector.tensor_copy(out=zeP[:, m2], in_=acc)
                    else:
                        nc.scalar.copy(out=zeP[:, m2], in_=acc)
                    nc.vector.bn_stats(out=stats[:, m2, p, 0], in_=zeP[:, m2, 0])
                    nc.vector.bn_stats(out=stats[:, m2, p, 1], in_=zeP[:, m2, 1])
                nc.sync.dma_start(out=zdst[:, p], in_=zeP)

            def zl_load(zsrc, p, nm):
                zl = zlp.tile([128, MC, 2, NB], F16, tag="zl",
                              name=f"zl{nm}_{p}")
                # alternate DMA queues so reloads aren't serialized on one
                (nc.scalar if p % 2 == 0 else nc.gpsimd).dma_start(
                    out=zl, in_=zsrc[:, p])
                return zl

            def h_from_z(zsrc, s_, t_, p, nm, zl_pre=None, veng_n=3):
                zl = zl_pre if zl_pre is not None else zl_load(zsrc, p, nm)
                h = hp.tile([128, MC, 2, NB], F16, tag="h", name=f"h{nm}_{p}")
                for m in range(MC):
                    apply_pair(h, m, zl[:, m], s_, t_, veng=(m < veng_n))
                return h

            def prefetch_zl(zsrc, nm, n=3):
                return [zl_load(zsrc, p, nm) for p in range(n)]

            def half_aggr(stats, nm):
                """Aggregate the first npair/2 pairs' records early (free --
                runs while the second half still computes)."""
                mvA = sp.tile([128, MC, 2], F32, name=f"mvA{nm}")
                for m in range(MC):
                    nc.vector.bn_aggr(out=mvA[:, m], in_=stats[:, m, :npair // 2])
                return mvA

            def finalize_stats(stats, g_s, bt_s, nm, mvA):
                mvB = sp.tile([128, MC, 2], F32, name=f"mvB{nm}")
                for m in range(MC):
                    nc.vector.bn_aggr(out=mvB[:, m], in_=stats[:, m, npair // 2:])
                # per-half counts are equal (bl/2): S1 = (bl/2)(mA+mB),
                # S2 = (bl/2)(vA+mA^2 + vB+mB^2)
                cci = sp.tile([128, MC, 2], F32, name=f"cci{nm}")
                tmp = sp.tile([128, MC], F32, name=f"tmq{nm}")
                tmpb = sp.tile([128, MC], F32, name=f"tmqb{nm}")
                nc.vector.tensor_mul(tmp, mvA[:, :, 0], mvA[:, :, 0])
                nc.vector.tensor_add(tmp, tmp, mvA[:, :, 1])
                nc.vector.tensor_mul(tmpb, mvB[:, :, 0], mvB[:, :, 0])
                nc.vector.tensor_add(tmpb, tmpb, mvB[:, :, 1])
                nc.vector.tensor_add(tmp, tmp, tmpb)
                nc.vector.tensor_scalar_mul(cci[:, :, 1], tmp, float(bl) / 2)
                nc.vector.tensor_add(tmpb, mvA[:, :, 0], mvB[:, :, 0])
                nc.vector.tensor_scalar_mul(cci[:, :, 0], tmpb, float(bl) / 2)
                di = dr.tile([128, MC * 2], F32, name=f"di{nm}")
                do_ = dr.tile([128, MC * 2], F32, name=f"do{nm}")
                nc.gpsimd.dma_start(out=di, in_=cci)
                nc.gpsimd.collective_compute(
                    "AllReduce", ALU.add,
                    replica_groups=[list(range(ncores))],
                    ins=[di.opt()], outs=[do_.opt()],
                )
                ccg = sp.tile([128, MC, 2], F32, name=f"ccg{nm}")
                nc.gpsimd.dma_start(out=ccg, in_=do_)
                meanv = sp.tile([128, MC], F32, name=f"mean{nm}")
                nc.vector.tensor_scalar_mul(meanv, ccg[:, :, 0], 1.0 / Bt)
                ex2 = sp.tile([128, MC], F32, name=f"ex2{nm}")
                nc.vector.tensor_scalar_mul(ex2, ccg[:, :, 1], 1.0 / Bt)
                vart = sp.tile([128, MC], F32, name=f"var{nm}")
                nc.vector.tensor_mul(vart, meanv, meanv)
                nc.vector.tensor_sub(vart, ex2, vart)
                sd = sp.tile([128, MC], F32, name=f"sd{nm}")
                nc.scalar.activation(out=sd, in_=vart, func=AF.Sqrt, bias=eps_t[:])
                rstd = sp.tile([128, MC], F32, name=f"rstd{nm}")
                nc.vector.reciprocal(out=rstd, in_=sd)
                s_ = sp.tile([128, MC], F32, name=f"s{nm}")
                nc.vector.tensor_mul(s_, g_s, rstd)
                t_ = sp.tile([128, MC], F32, name=f"t{nm}")
                nc.vector.tensor_mul(t_, meanv, s_)
                nc.vector.tensor_sub(t_, bt_s, t_)
                return s_, t_

            # =========================================================
            # Layers 1+2 (software-pipelined: l1 runs 2 pairs ahead)
            # =========================================================
            st2 = stp.tile([128, MC, npair, 2, 6], F32, tag="st", name="st2")
            hq = [l1_pair(0), l1_pair(1)]
            mvA2 = None
            for p in range(npair):
                if p + 2 < npair:
                    hq.append(l1_pair(p + 2))
                mm_pair(w2s, hq[p], st2, z2buf, p, "2", last=(p == npair - 1))
                if p == npair // 2 - 1:
                    mvA2 = half_aggr(st2, "2")
            pre3 = prefetch_zl(z2buf, "3")
            # W3 reuses W2's buffer; its load lands in the AR2 window
            w3s = wq.tile([128, MC, H], F16, tag="w", name="w3s")
            nc.sync.dma_start(out=w3s, in_=w3h_in)
            s2, t2p = finalize_stats(st2, g2s, bt2s, "2", mvA2)

            # ---- Layer 3 ----
            st3 = stp.tile([128, MC, npair, 2, 6], F32, tag="st", name="st3")
            mvA3 = None
            for p in range(npair):
                h = h_from_z(z2buf, s2, t2p, p, "3",
                             zl_pre=pre3[p] if p < len(pre3) else None)
                mm_pair(w3s, h, st3, z3buf, p, "3", last=(p == npair - 1))
                if p == npair // 2 - 1:
                    mvA3 = half_aggr(st3, "3")
            pre4 = prefetch_zl(z3buf, "4")
            s3, t3p = finalize_stats(st3, g3s, bt3s, "3", mvA3)
            # fold s3 into W4 rows: h3' = relu(z3 + t3/s3), W4' = s3*W4
            rs3 = sp.tile([128, MC], F32)
            nc.vector.reciprocal(out=rs3, in_=s3)
            t3d = sp.tile([128, MC], F32)
            nc.vector.tensor_mul(t3d, t3p, rs3)
            for k in range(MC):
                nc.vector.tensor_scalar_mul(w4s[:, k], w4s[:, k], s3[:, k:k + 1])

            # =========================================================
            # Layer 4 -> theta, transposed on-chip via PE
            # =========================================================
            for p in range(npair):
                h3 = h_from_z(z3buf, None, t3d, p, "4",
                              zl_pre=pre4[p] if p < len(pre4) else None,
                              veng_n=5)
                thp = psB.tile([3, 2, NB], F32, tag="pB", name=f"thp{p}")
                for k in range(MC):
                    nc.tensor.matmul(thp[:, 0], w4s[:, k], h3[:, k, 0],
                                     start=(k == 0), stop=(k == MC - 1))
                    nc.tensor.matmul(thp[:, 1], w4s[:, k], h3[:, k, 1],
                                     start=(k == 0), stop=(k == MC - 1))
                ths = xp.tile([3, 2, NB], F16, tag="ths", name=f"ths{p}")
                nc.scalar.activation(out=ths, in_=thp, func=AF.Identity,
                                     bias=b4s[:], scale=1.0)
                for j in range(8):
                    tps = psA.tile([128, 3], F16, tag="pA", name=f"tps{p}_{j}")
                    nc.tensor.transpose(
                        tps[:], ths[:, j // 4, (j % 4) * 128:(j % 4 + 1) * 128],
                        eye3h)
                    nc.vector.tensor_copy(out=thn[:, 8 * p + j, :], in_=tps)
            nc.sync.dma_start(
                out=theta_out.rearrange("(p t) f -> p t f", p=128), in_=thn)

            # =========================================================
            # Forward kinematics on thn (batch on partitions x nt free)
            # =========================================================
            def trig(src, shift, nm):
                w = sp.tile([128, nt], F32, name=f"w{nm}")
                nc.vector.add_range_wrap(out=w, in_=src, shift=shift,
                                         bound=PI, period=2 * PI)
                o = sp.tile([128, nt], F32, name=f"o{nm}")
                nc.scalar.activation(out=o, in_=w, func=AF.Sin, bias=zero128[:])
                return o

            th0 = thn[:, :, 0]
            th1 = thn[:, :, 1]
            th2 = thn[:, :, 2]
            t12 = sp.tile([128, nt], F32, name="t12")
            nc.vector.tensor_add(t12, th1, th2)
            s0v = trig(th0, 0.0, "s0")
            c0v = trig(th0, PI / 2, "c0")
            s1v = trig(th1, 0.0, "s1v")
            c1v = trig(th1, PI / 2, "c1v")
            s12v = trig(t12, 0.0, "s12")
            c12v = trig(t12, PI / 2, "c12")

            Lt = sp.tile([128, nt], F32, name="Lt")
            nc.vector.tensor_scalar_mul(Lt, c12v, 0.115)
            nc.vector.scalar_tensor_tensor(out=Lt, in0=c1v, scalar=0.12, in1=Lt,
                                           op0=ALU.mult, op1=ALU.add)
            pzt = sp.tile([128, nt], F32, name="pzt")
            nc.vector.tensor_scalar_mul(pzt, s12v, 0.115)
            nc.vector.scalar_tensor_tensor(out=pzt, in0=s1v, scalar=0.12, in1=pzt,
                                           op0=ALU.mult, op1=ALU.add)
            predn = sp.tile([128, nt, 3], F32, name="predn")
            nc.vector.tensor_mul(predn[:, :, 0], c0v, Lt)
            nc.vector.tensor_mul(predn[:, :, 1], s0v, Lt)
            nc.vector.tensor_copy(out=predn[:, :, 2], in_=pzt)
            nc.sync.dma_start(
                out=pred_out.rearrange("(p t) f -> p t f", p=128), in_=predn)

    nc.compile()
    return nc


def _get_module():
    global _MODULE
    if _MODULE is None:
        _MODULE = _build_module()
    return _MODULE


def kernel(x, W1, b1, g1, bt1, W2, b2, g2, bt2, W3, b3, g3, bt3, W4, b4,
           **run_kwargs):
    nc = _get_module()
    x = np.asarray(x, dtype=np.float32)
    x16 = x.astype(np.float16)
    # full batch + ones col, packed [128, 2, B/256, 4] (any row->slot
    # assignment works -- only sums are taken)
    xa_full = np.concatenate(
        [x16, np.ones((x16.shape[0], 1), np.float16)], axis=1)
    xf4 = np.ascontiguousarray(
        xa_full.reshape(2, B // 256, 128, 4).transpose(2, 0, 1, 3))

    def prearr_w(W):
        W = np.asarray(W, np.float32).astype(np.float16)
        return np.ascontiguousarray(
            W.reshape(MC, 128, -1).transpose(1, 0, 2))

    def prearr_p(v):
        return np.ascontiguousarray(
            np.asarray(v, np.float32).reshape(MC, 128).T)

    shared = {
        "xf4": xf4,
        "w1h": np.ascontiguousarray(np.asarray(W1, np.float32).astype(np.float16)),
        "w1f": np.ascontiguousarray(np.asarray(W1, np.float32)),
        "w2h": prearr_w(W2),
        "w3h": prearr_w(W3),
        "w4h": prearr_w(W4),
        "g1v": prearr_p(g1),
        "bt1v": prearr_p(bt1),
        "g2v": prearr_p(g2),
        "bt2v": prearr_p(bt2),
        "g3v": prearr_p(g3),
        "bt3v": prearr_p(bt3),
        "b4v": np.ascontiguousarray(np.asarray(b4, np.float32).reshape(3, 1)),
        "eye3": np.eye(3, dtype=np.float32),
    }
    in_maps = []
    for i in range(N_CORES):
        xs = x[i * BL:(i + 1) * BL]
        # permuted transposed shard: column c*128+p holds shard row p*128+c
        xt_p = xs.T.astype(np.float16).reshape(3, 128, BL // 128) \
            .swapaxes(1, 2).reshape(3, BL)
        m = dict(shared)
        m["xt"] = np.ascontiguousarray(xt_p)
        in_maps.append(m)
    res = run_bass_kernel_spmd(nc, in_maps, core_ids=list(range(N_CORES)),
                               **run_kwargs)
    theta = np.concatenate([res.results[i]["theta"] for i in range(N_CORES)], axis=0)
    pred = np.concatenate([res.results[i]["pred"] for i in range(N_CORES)], axis=0)
    kernel.last_results = res
    return theta.astype(np.float32), pred.astype(np.float32)
